# revision 10
# baseline (speedup 1.0000x reference)
"""Trainium2 Bass kernel for nn_AttentionHead (B=8, S=4096, D=128).

Sharding: data-parallel over the batch dim — 1 batch element per NeuronCore,
8 cores, SPMD (same NEFF, different x slice), weights replicated. No
collectives.

Per-core pipeline (S=4096 seq, D=128 head dim, all-on-chip):
  1. x [4096,128] f32 -> cast-load f16 -> PE-transpose -> xT [d, s] f16
  2. q/k/v projections: matmul(lhsT=xT s-tile, rhs=W^T) -> PSUM f32
     - LN stats (bn_stats/bn_aggr) straight off PSUM, apply (x-mu)*rstd via
       one ACT op (per-partition scale/bias), PE-transpose to [h, s] and
       fold LN weight/bias in the PSUM->SBUF copy (per-partition h there).
  3. attention per 128-query i-tile:
     - scores = qT_tile^T @ kT (N=512 matmuls into a 2-bank PSUM tile)
     - exp via ACT directly off PSUM with scale=1/sqrt(D); the softmax
       row-sum comes free via ACT accum_out (free-dim = keys). No max
       subtraction: scores are ~N(0,1) (LN'd q,k), exp stays in f16 range.
     - DMA-xbar-transpose exp [i,j] -> expT [j,i] (f16, 1MB per i-tile)
     - PV: out[i,h] accumulates 32 matmuls lhsT=expT chunk, rhs=v chunk
       into one PSUM tile; normalize by 1/rowsum (per-partition scalar) in
       the PSUM->SBUF copy; DMA out.

All SBUF pools stay open for the whole kernel (no SBUF slot reuse across
phases): SBUF-space reuse attaches release waits to the DMAs that load into
recycled space, and walrus rejects DMAs with more than a couple of sync
waits ("Too many sync wait commands"). Only PSUM pools are scoped.
"""

import math

import numpy as np

import concourse.bass as bass
from concourse import bacc
import concourse.mybir as mybir
import concourse.tile as tile
from concourse.bass_utils import run_bass_kernel_spmd
from concourse.masks import make_identity

F16 = mybir.dt.float16
F32 = mybir.dt.float32
AF = mybir.ActivationFunctionType
ALU = mybir.AluOpType

B, S, D = 8, 4096, 128
P = 128
NT = S // P  # 32 s-tiles
EPS = 1e-5
ISQRT_D = 1.0 / math.sqrt(D)
N_CORES = 8
JC = 1024  # key-chunk width for the exp pass (2 PSUM banks)
NJC = S // JC


def _ln_param_to_sbuf(nc, pool, dram_ap, tag):
    t = pool.tile([P, 1], F32, tag=tag)
    nc.sync.dma_start(t, dram_ap[:, None])
    return t


def _build_attention(tc, out_d, x_d, w_d, ln_d):
    """Emit the single-core attention program.

    out_d: [S, D] f32 output AP.  x_d: [S, D] f32 input AP.
    w_d: dict q/k/v -> [D, D] f32 weight AP (torch Linear layout: out = x @ W^T).
    ln_d: dict qw/qb/kw/kb -> [D] f32 LN param APs.
    """
    nc = tc.nc

    with (
        tc.tile_pool(name="const", bufs=1) as const,
        tc.tile_pool(name="big", bufs=1) as big,
        tc.tile_pool(name="wtmp", bufs=3) as wtmp,
        tc.tile_pool(name="xload", bufs=1) as xload,
        tc.tile_pool(name="stat", bufs=6) as stat,
        tc.tile_pool(name="stage", bufs=4) as stage,
        tc.tile_pool(name="attn", bufs=2) as attn,
        tc.tile_pool(name="small", bufs=4) as small,
    ):
        ident = const.tile([P, P], F16)
        make_identity(nc, ident)
        epst = const.tile([P, 1], F32)
        nc.vector.memset(epst, EPS)
        qnw = _ln_param_to_sbuf(nc, const, ln_d["qw"], "qnw")
        qnb = _ln_param_to_sbuf(nc, const, ln_d["qb"], "qnb")
        knw = _ln_param_to_sbuf(nc, const, ln_d["kw"], "knw")
        knb = _ln_param_to_sbuf(nc, const, ln_d["kb"], "knb")

        # --- weights: load [h,d] f32, cast f16, PE-transpose -> W^T [d,h] f16
        WT = {}
        with tc.tile_pool(name="wps", bufs=3, space="PSUM") as wps:
            for name in ("q", "k", "v"):
                w32 = wtmp.tile([P, P], F32, tag=f"w32_{name}")
                nc.sync.dma_start(w32, w_d[name])
                w16 = wtmp.tile([P, P], F16, tag=f"w16_{name}")
                nc.vector.tensor_copy(w16, w32)
                ps = wps.tile([P, P], F16, tag="wtp")
                nc.tensor.transpose(ps, w16, ident)
                wt = const.tile([P, P], F16, tag=f"wt_{name}")
                nc.scalar.activation(wt, ps, AF.Copy)
                WT[name] = wt

        # --- x load (cast f32->f16 in the SWDGE DMA) + PE-transpose to xT
        xT = big.tile([P, NT, P], F16, tag="xT")  # [d, t, s%128]
        x16 = xload.tile([P, NT, P], F16)  # [s%128, t, d]
        nc.gpsimd.dma_start(x16, x_d.rearrange("(t p) d -> p t d", p=P))
        with tc.tile_pool(name="xps", bufs=4, space="PSUM") as xps:
            for t in range(NT):
                ps = xps.tile([P, P], F16, tag="xtp")
                nc.tensor.transpose(ps, x16[:, t, :], ident)
                nc.scalar.activation(xT[:, t, :], ps, AF.Copy)

        # --- projections + layernorm -> qT, kT [h, s] f16; v [s, h] f16
        qT = big.tile([P, NT, P], F16, tag="qT")
        kT = big.tile([P, NT, P], F16, tag="kT")
        v16 = big.tile([P, NT, P], F16, tag="v16")
        with (
            tc.tile_pool(name="pps", bufs=2, space="PSUM") as pps,
            tc.tile_pool(name="tps", bufs=2, space="PSUM") as tps,
        ):
            for t in range(NT):
                proj = {}
                for name in ("q", "k", "v"):
                    ps = pps.tile([P, P], F32, tag=f"p_{name}")
                    nc.tensor.matmul(ps, lhsT=xT[:, t, :], rhs=WT[name],
                                     start=True, stop=True)
                    proj[name] = ps
                # v: plain copy/cast to SBUF
                nc.scalar.activation(v16[:, t, :], proj["v"], AF.Copy)
                for name, Tdst, wsb, bsb in (
                    ("q", qT, qnw, qnb),
                    ("k", kT, knw, knb),
                ):
                    ps = proj[name]
                    st = stat.tile([P, 6], F32, tag="st")
                    nc.vector.bn_stats(st, ps)
                    mv = stat.tile([P, 2], F32, tag="mv")
                    nc.vector.bn_aggr(mv, st)
                    rstd = stat.tile([P, 1], F32, tag="rstd")
                    nc.scalar.activation(rstd, mv[:, 1:2], AF.Sqrt, bias=epst)
                    nc.vector.reciprocal(rstd, rstd)
                    nmr = stat.tile([P, 1], F32, tag="nmr")
                    # (-mu) * rstd
                    nc.vector.scalar_tensor_tensor(
                        nmr, in0=mv[:, 0:1], scalar=-1.0, in1=rstd,
                        op0=ALU.mult, op1=ALU.mult)
                    s1 = stage.tile([P, P], F16, tag=f"s1_{name}")
                    # (x - mu) * rstd, PSUM -> SBUF f16
                    nc.scalar.activation(s1, ps, AF.Identity, scale=rstd, bias=nmr)
                    tp = tps.tile([P, P], F16, tag="tp")
                    nc.tensor.transpose(tp, s1, ident)
                    # * ln_w + ln_b (per-partition = h after transpose)
                    nc.scalar.activation(Tdst[:, t, :], tp, AF.Identity,
                                         scale=wsb, bias=bsb)

        # --- attention
        kT2 = kT.rearrange("h t s -> h (t s)")
        with (
            tc.tile_pool(name="scps", bufs=2, space="PSUM") as scps,
            tc.tile_pool(name="pvps", bufs=2, space="PSUM") as pvps,
        ):
            for i in range(NT):
                exp_nat = attn.tile([P, S], F16, tag="expn")  # [i, j]
                parts = small.tile([P, NJC], F32, tag="parts")
                for jc in range(NJC):
                    sc = scps.tile([P, JC], F32, tag="sc")
                    for h in range(JC // 512):
                        nc.tensor.matmul(
                            sc[:, h * 512:(h + 1) * 512], lhsT=qT[:, i, :],
                            rhs=kT2[:, jc * JC + h * 512:jc * JC + (h + 1) * 512],
                            start=True, stop=True)
                    nc.scalar.activation(
                        exp_nat[:, jc * JC:(jc + 1) * JC], sc, AF.Exp,
                        scale=ISQRT_D, accum_out=parts[:, jc:jc + 1])
                sums = small.tile([P, 1], F32, tag="sums")
                nc.vector.reduce_sum(sums, parts, axis=mybir.AxisListType.X)
                rsum = small.tile([P, 1], F32, tag="rsum")
                nc.vector.reciprocal(rsum, sums)
                expT = attn.tile([P, NT, P], F16, tag="expt")  # [j%128, jt, i]
                nc.sync.dma_start_transpose(expT, exp_nat)
                ops = pvps.tile([P, P], F32, tag="pv")
                for c in range(NT):
                    nc.tensor.matmul(ops, lhsT=expT[:, c, :], rhs=v16[:, c, :],
                                     start=(c == 0), stop=(c == NT - 1))
                osb = small.tile([P, P], F32, tag="osb")
                nc.vector.tensor_scalar_mul(osb, ops, rsum)
                nc.sync.dma_start(out_d[i * P:(i + 1) * P, :], osb)


_NC_CACHE = None


def _build():
    global _NC_CACHE
    if _NC_CACHE is not None:
        return _NC_CACHE
    nc = bacc.Bacc("TRN2", target_bir_lowering=False, debug=False)
    x = nc.dram_tensor("x", [S, D], F32, kind="ExternalInput").ap()
    wq = nc.dram_tensor("Wq", [D, D], F32, kind="ExternalInput").ap()
    wk = nc.dram_tensor("Wk", [D, D], F32, kind="ExternalInput").ap()
    wv = nc.dram_tensor("Wv", [D, D], F32, kind="ExternalInput").ap()
    qn_w = nc.dram_tensor("qn_w", [D], F32, kind="ExternalInput").ap()
    qn_b = nc.dram_tensor("qn_b", [D], F32, kind="ExternalInput").ap()
    kn_w = nc.dram_tensor("kn_w", [D], F32, kind="ExternalInput").ap()
    kn_b = nc.dram_tensor("kn_b", [D], F32, kind="ExternalInput").ap()
    out = nc.dram_tensor("out", [S, D], F32, kind="ExternalOutput").ap()
    with tile.TileContext(nc) as tc:
        _build_attention(
            tc, out, x,
            {"q": wq, "k": wk, "v": wv},
            {"qw": qn_w, "qb": qn_b, "kw": kn_w, "kb": kn_b},
        )
    nc.compile()
    _NC_CACHE = nc
    return nc


def kernel(x, Wq, Wk, Wv, qn_w, qn_b, kn_w, kn_b, _run_kwargs=None):
    nc = _build()
    x = np.asarray(x, dtype=np.float32)
    shared = {
        "Wq": np.ascontiguousarray(np.asarray(Wq, np.float32)),
        "Wk": np.ascontiguousarray(np.asarray(Wk, np.float32)),
        "Wv": np.ascontiguousarray(np.asarray(Wv, np.float32)),
        "qn_w": np.ascontiguousarray(np.asarray(qn_w, np.float32)),
        "qn_b": np.ascontiguousarray(np.asarray(qn_b, np.float32)),
        "kn_w": np.ascontiguousarray(np.asarray(kn_w, np.float32)),
        "kn_b": np.ascontiguousarray(np.asarray(kn_b, np.float32)),
    }
    in_maps = [
        {"x": np.ascontiguousarray(x[b]), **shared} for b in range(B)
    ]
    res = run_bass_kernel_spmd(nc, in_maps, core_ids=list(range(N_CORES)),
                               **(_run_kwargs or {}))
    out = np.stack([res.results[b]["out"] for b in range(B)], axis=0)
    if _run_kwargs:
        kernel.last_results = res
    return out.astype(np.float32)
